# revision 1
# baseline (speedup 1.0000x reference)
"""Trainium2 Bass kernel for the GaussianModel occupancy-grid problem.

Strategy
--------
occ[p] = sum_g w(g, block) * exp(power(p, g)) where power is a quadratic
form in the voxel coordinate p.  We rewrite power as a rank-10 inner
product  Phi(p) . c_g  (6 quadratic + 3 linear + 1 const feature) and fold
the per-(gaussian, block) weight w into the constant coefficient as
log(w)  (w == 0  ->  -1e10, which underflows exp to exactly 0).

Only ~1.5% of (gaussian, block) pairs pass the reference's box test, so the
host compacts, per pair of z-adjacent blocks (128 voxels = full partition
dim), the union of active gaussians and ships a [11, U] coefficient matrix
(9 shared features + 2 per-block constant rows selected by indicator
features).  The device then runs, per work item:

    PE:   power = lhsT(11x128)^T @ rhs(11xL)  -> PSUM   (float32r)
    ACT:  one big exp over each packed PSUM batch -> SBUF
    DVE:  segmented reduce over gaussians -> val column

Work items are packed into [128, 1024] PSUM batches (2 banks), the exp is
one activation op per batch, and same-length contiguous items share one 3D
tensor_reduce.  All 8 cores run one SPMD program; a common descending
length schedule (elementwise max across cores) keeps shapes identical.

Coordinates are re-centered per pair (p' = p - pair_center) to avoid
catastrophic cancellation in the expanded quadratic.
"""

import numpy as np

NB = 16          # num_blocks
RES = 64         # resolution
SPLIT = 4        # voxels per block side
N_CORES = 8
K_FEAT = 11      # 9 shared features + 2 block-indicator/const rows
PSUM_COLS = 1024  # 2 banks per batch
BANKS = 2
BANK_COLS = 512
MAX_CHUNK = 512  # matmul free-dim limit (single PSUM bank)

_CACHE = {}


def _host_prep(_xyz, _scaling, _rotation, _opacity):
    """Mirror of the reference's per-gaussian preprocessing (numpy fp32)."""
    f32 = np.float32
    opac = (1.0 / (1.0 + np.exp(-_opacity[:, 0].astype(f32)))).astype(f32)
    keep = opac > 0.005
    opa = np.where(keep, opac, f32(0.0)).astype(f32)

    BIG = f32(1e10)
    mn = np.min(np.where(keep[:, None], _xyz, BIG), axis=0)
    mx = np.max(np.where(keep[:, None], _xyz, -BIG), axis=0)
    center = ((mn + mx) / 2).astype(f32)
    scale = (f32(1.8) / np.max(mx - mn)).astype(f32)
    xyzs = ((_xyz - center) * scale).astype(f32)
    stds = (np.exp(_scaling) * scale).astype(f32)

    q = (_rotation / np.linalg.norm(_rotation, axis=1, keepdims=True)).astype(f32)
    r, x, y, z = q[:, 0], q[:, 1], q[:, 2], q[:, 3]
    R = np.stack([
        np.stack([1 - 2 * (y * y + z * z), 2 * (x * y - r * z), 2 * (x * z + r * y)], -1),
        np.stack([2 * (x * y + r * z), 1 - 2 * (x * x + z * z), 2 * (y * z - r * x)], -1),
        np.stack([2 * (x * z - r * y), 2 * (y * z + r * x), 1 - 2 * (x * x + y * y)], -1),
    ], axis=1).astype(f32)
    L = R * stds[:, None, :]
    C = np.einsum('nij,nkj->nik', L, L).astype(f32)
    a, b, c = C[:, 0, 0], C[:, 0, 1], C[:, 0, 2]
    d, e, f = C[:, 1, 1], C[:, 1, 2], C[:, 2, 2]
    inv_det = (1.0 / (a * d * f + 2 * e * c * b - e * e * a - c * c * d
                      - b * b * f + 1e-24)).astype(f32)
    ia = ((d * f - e * e) * inv_det).astype(f32)
    ib = ((e * c - b * f) * inv_det).astype(f32)
    ic = ((e * b - c * d) * inv_det).astype(f32)
    id_ = ((a * f - c * c) * inv_det).astype(f32)
    ie = ((b * c - e * a) * inv_det).astype(f32)
    if_ = ((a * d - b * b) * inv_det).astype(f32)

    logopa = np.where(opa > 0, np.log(np.maximum(opa, 1e-30)),
                      f32(-1e10)).astype(f32)
    return xyzs, opa, logopa, (ia, ib, ic, id_, ie, if_)


def _build_workload(xyzs, opa, logopa, inv, banks=BANKS):
    """Enumerate per-pair active unions and build per-core work items.

    Returns (schedule, per_core_items) where schedule is identical across
    cores and per_core_items[c] is a list aligned with the schedule ranks.
    """
    f32 = np.float32
    ia, ib, ic, id_, ie, if_ = inv
    lin = np.linspace(-1.0, 1.0, RES).astype(f32)
    relax = f32((2.0 / NB) * 1.5)
    gx, gy, gz = xyzs[:, 0], xyzs[:, 1], xyzs[:, 2]
    act = opa > 0

    vmin = lin[np.arange(NB) * SPLIT] - relax
    vmax = lin[np.arange(NB) * SPLIT + SPLIT - 1] + relax
    Fx = (gx[None, :] > vmin[:, None]) & (gx[None, :] < vmax[:, None])
    Fy = (gy[None, :] > vmin[:, None]) & (gy[None, :] < vmax[:, None])
    Fz = (gz[None, :] > vmin[:, None]) & (gz[None, :] < vmax[:, None])
    Fz_pair = (Fz & act).reshape(NB // 2, 2, -1)

    chunks = []  # (length, bi, bj, m, gauss-index-array)
    for bi in range(NB):
        fx = Fx[bi]
        for bj in range(NB):
            fxy = fx & Fy[bj] & act
            if not fxy.any():
                continue
            for m in range(NB // 2):
                un = fxy & (Fz_pair[m, 0] | Fz_pair[m, 1])
                idx = np.nonzero(un)[0]
                for s in range(0, idx.size, MAX_CHUNK):
                    part = idx[s:s + MAX_CHUNK]
                    chunks.append((part.size, bi, bj, m, part))
    chunks.sort(key=lambda t: -t[0])

    m_items = (len(chunks) + N_CORES - 1) // N_CORES
    # deal round-robin: chunk k -> core k % 8, rank k // 8.  The schedule
    # length per rank is the max over cores (= the first core's, since
    # lengths are descending), rounded up to a multiple of 4.
    l_sched = []
    for rank in range(m_items):
        lmax = chunks[rank * N_CORES][0]
        l_sched.append(max(4, (lmax + 3) // 4 * 4))

    per_core = [[None] * m_items for _ in range(N_CORES)]
    for k, ch in enumerate(chunks):
        per_core[k % N_CORES][k // N_CORES] = ch

    # PSUM batch packing (first fit into `banks` banks of 512 cols)
    placed = []          # (batch, offset) per rank
    batches = [[]]       # list of ranks per batch
    fill = [0] * banks
    for rank, L in enumerate(l_sched):
        spot = None
        for bk in range(banks):
            if fill[bk] + L <= BANK_COLS:
                spot = bk
                break
        if spot is None:
            batches.append([])
            fill = [0] * banks
            spot = 0
        placed.append((len(batches) - 1, spot * BANK_COLS + fill[spot]))
        fill[spot] += L
        batches[-1].append(rank)

    # reduce chains: runs of same-L, contiguous-offset, same-batch ranks
    chains = []  # (rank0, n, L, batch, offset)
    i = 0
    while i < m_items:
        b0, o0 = placed[i]
        L = l_sched[i]
        j = i
        while (j + 1 < m_items and l_sched[j + 1] == L
               and placed[j + 1][0] == b0
               and placed[j + 1][1] == o0 + (j + 1 - i) * L):
            j += 1
        chains.append((i, j - i + 1, L, b0, o0))
        i = j + 1

    cum = np.concatenate([[0], np.cumsum(l_sched)]).astype(np.int64)
    schedule = {
        "m_items": m_items,
        "l_sched": l_sched,
        "cum": cum,
        "tot_cols": int(cum[-1]),
        "placed": placed,
        "batches": batches,
        "chains": chains,
        "banks": banks,
    }
    return schedule, per_core


def _build_inputs(schedule, per_core, xyzs, logopa, inv):
    """Build per-core LHS [M,11,128] / RHS [11,TOT] arrays + assembly map."""
    f32 = np.float32
    ia, ib, ic, id_, ie, if_ = inv
    lin = np.linspace(-1.0, 1.0, RES).astype(f32)
    relax = f32((2.0 / NB) * 1.5)
    gx, gy, gz = xyzs[:, 0], xyzs[:, 1], xyzs[:, 2]

    m_items = schedule["m_items"]
    l_sched = schedule["l_sched"]
    cum = schedule["cum"]
    tot = schedule["tot_cols"]

    NEG = f32(-1e10)
    in_maps = []
    assembly = []  # (core, rank, bi, bj, m) for every real chunk
    for c in range(N_CORES):
        LHS = np.zeros((m_items, K_FEAT, 128), f32)
        RHS = np.zeros((K_FEAT, tot), f32)
        RHS[9:11, :] = NEG  # default: padding columns contribute exp(-1e10)=0
        for rank in range(m_items):
            ch = per_core[c][rank]
            Lr = l_sched[rank]
            if ch is None:
                # dummy item: indicator rows only; all columns padded
                LHS[rank, 9, :64] = 1.0
                LHS[rank, 10, 64:] = 1.0
                continue
            _, bi, bj, m, idx = ch
            xs = lin[bi * 4:bi * 4 + 4]
            ys = lin[bj * 4:bj * 4 + 4]
            zs = lin[8 * m:8 * m + 8]  # both blocks' z coords
            cx = f32((xs[0] + xs[3]) * 0.5)
            cy = f32((ys[0] + ys[3]) * 0.5)
            cz = f32((zs[0] + zs[7]) * 0.5)
            xl, yl, zl = xs - cx, ys - cy, zs - cz

            # 128 points: block0 = zl[0:4], block1 = zl[4:8];
            # within a block: p = ix*16 + iy*4 + iz
            X, Y, Z = np.meshgrid(xl, yl, zl[:4], indexing='ij')
            P0 = np.stack([X.ravel(), Y.ravel(), Z.ravel()], 0)
            X, Y, Z = np.meshgrid(xl, yl, zl[4:], indexing='ij')
            P1 = np.stack([X.ravel(), Y.ravel(), Z.ravel()], 0)
            P = np.concatenate([P0, P1], 1).astype(f32)  # [3, 128]
            px, py, pz = P[0], P[1], P[2]
            LHS[rank, 0] = px * px
            LHS[rank, 1] = py * py
            LHS[rank, 2] = pz * pz
            LHS[rank, 3] = px * py
            LHS[rank, 4] = px * pz
            LHS[rank, 5] = py * pz
            LHS[rank, 6] = px
            LHS[rank, 7] = py
            LHS[rank, 8] = pz
            LHS[rank, 9, :64] = 1.0
            LHS[rank, 10, 64:] = 1.0

            # gaussians (local coords)
            g0x = (gx[idx] - cx).astype(f32)
            g0y = (gy[idx] - cy).astype(f32)
            g0z = (gz[idx] - cz).astype(f32)
            A_ = ia[idx]; B_ = ib[idx]; Cc = ic[idx]
            D_ = id_[idx]; E_ = ie[idx]; F_ = if_[idx]
            Agx = A_ * g0x + B_ * g0y + Cc * g0z
            Agy = B_ * g0x + D_ * g0y + E_ * g0z
            Agz = Cc * g0x + E_ * g0y + F_ * g0z
            const = (-0.5 * (g0x * Agx + g0y * Agy + g0z * Agz)).astype(f32)

            o = cum[rank]
            n = idx.size
            RHS[0, o:o + n] = -0.5 * A_
            RHS[1, o:o + n] = -0.5 * D_
            RHS[2, o:o + n] = -0.5 * F_
            RHS[3, o:o + n] = -B_
            RHS[4, o:o + n] = -Cc
            RHS[5, o:o + n] = -E_
            RHS[6, o:o + n] = Agx
            RHS[7, o:o + n] = Agy
            RHS[8, o:o + n] = Agz
            # per-block constant rows: -1e10 where not active in that block
            in_b0 = ((gz[idx] > lin[8 * m] - relax)
                     & (gz[idx] < lin[8 * m + 3] + relax))
            in_b1 = ((gz[idx] > lin[8 * m + 4] - relax)
                     & (gz[idx] < lin[8 * m + 7] + relax))
            base = const + logopa[idx]
            RHS[9, o:o + n] = np.where(in_b0, base, NEG)
            RHS[10, o:o + n] = np.where(in_b1, base, NEG)
            assembly.append((c, rank, bi, bj, m))
        in_maps.append({"lhs": LHS, "rhs": RHS})
    return in_maps, assembly


def _build_program(schedule, opts=None):
    import concourse.bass as bass  # noqa: F401
    import concourse.bacc as bacc
    import concourse.tile as tile
    import concourse.mybir as mybir

    opts = opts or {}
    no_mm = opts.get("no_mm", False)
    no_act = opts.get("no_act", False)
    no_red = opts.get("no_red", False)
    mm_dtype = opts.get("mm_dtype", "float32r")

    m_items = schedule["m_items"]
    l_sched = schedule["l_sched"]
    cum = schedule["cum"]
    tot = schedule["tot_cols"]
    placed = schedule["placed"]
    batches = schedule["batches"]
    chains = schedule["chains"]
    banks = schedule.get("banks", BANKS)
    psum_cols = banks * BANK_COLS
    psum_bufs = max(2, 8 // banks)
    stg_bufs = opts.get("stg_bufs", psum_bufs + 1)
    f32 = mybir.dt.float32

    mmdt = getattr(mybir.dt, mm_dtype)

    nc = bacc.Bacc("TRN2", target_bir_lowering=False, debug=False,
                   num_devices=N_CORES)
    lhs_d = nc.dram_tensor("lhs", [m_items, K_FEAT, 128], mmdt,
                           kind="ExternalInput")
    rhs_d = nc.dram_tensor("rhs", [K_FEAT, tot], mmdt, kind="ExternalInput")
    val_d = nc.dram_tensor("val", [128, m_items], f32, kind="ExternalOutput")

    # chains grouped by batch for emission order
    chains_by_batch = {}
    for ch in chains:
        chains_by_batch.setdefault(ch[3], []).append(ch)

    with tile.TileContext(nc) as tc:
        with tc.tile_pool(name="inp", bufs=1) as inp, \
             tc.tile_pool(name="stg", bufs=stg_bufs) as stg, \
             tc.tile_pool(name="vp", bufs=1) as vp, \
             tc.tile_pool(name="ps", bufs=psum_bufs, space="PSUM") as ps:
            lhs_t = inp.tile([K_FEAT, m_items * 128], mmdt, name="lhs_sb")
            rhs_t = inp.tile([K_FEAT, tot], mmdt, name="rhs_sb")
            val_t = vp.tile([128, m_items], f32, name="val_sb")

            # split input DMAs so early batches don't wait on the full load
            lhs_src = lhs_d.ap().rearrange("i k p -> k i p")
            lhs_dst = lhs_t.rearrange("k (i p) -> k i p", p=128)
            nsp = 4
            step = (m_items + nsp - 1) // nsp
            for s in range(0, m_items, step):
                e = min(s + step, m_items)
                nc.sync.dma_start(out=lhs_dst[:, s:e, :], in_=lhs_src[:, s:e, :])
            cstep = (tot + nsp - 1) // nsp
            cstep = (cstep + 3) // 4 * 4
            for s in range(0, tot, cstep):
                e = min(s + cstep, tot)
                nc.sync.dma_start(out=rhs_t[:, s:e], in_=rhs_d.ap()[:, s:e])

            for b, ranks in enumerate(batches):
                pt = ps.tile([128, psum_cols], f32, name=f"pt{b}", tag="pt")
                if not no_mm:
                    for rank in ranks:
                        L = l_sched[rank]
                        ofs = placed[rank][1]
                        nc.tensor.matmul(
                            pt[:, ofs:ofs + L],
                            lhs_t[:, rank * 128:(rank + 1) * 128],
                            rhs_t[:, int(cum[rank]):int(cum[rank]) + L],
                            start=True, stop=True)
                st = stg.tile([128, psum_cols], f32, name=f"st{b}", tag="st")
                if not no_act:
                    nc.scalar.activation(st, pt,
                                         mybir.ActivationFunctionType.Exp)
                if not no_red:
                    for (r0, n, L, _b, ofs) in chains_by_batch.get(b, []):
                        seg = st[:, ofs:ofs + n * L]
                        if n > 1:
                            seg = seg.rearrange("p (n l) -> p n l", l=L)
                        nc.vector.tensor_reduce(
                            val_t[:, r0:r0 + n], seg,
                            axis=mybir.AxisListType.X, op=mybir.AluOpType.add)
            if not no_red:
                nc.sync.dma_start(out=val_d.ap(), in_=val_t)
            else:
                nc.vector.memset(val_t, 0.0)
                nc.sync.dma_start(out=val_d.ap(), in_=val_t)

    nc.compile()
    return nc


def _assemble(schedule, assembly, results):
    occ = np.zeros((RES, RES, RES), np.float32)
    acc = {}
    for (c, rank, bi, bj, m) in assembly:
        v = results[c]["val"][:, rank]
        key = (bi, bj, m)
        if key in acc:
            acc[key] = acc[key] + v
        else:
            acc[key] = v.copy()
    for (bi, bj, m), v in acc.items():
        v = v.reshape(2, 4, 4, 4)
        occ[bi * 4:bi * 4 + 4, bj * 4:bj * 4 + 4, 8 * m:8 * m + 4] = v[0]
        occ[bi * 4:bi * 4 + 4, bj * 4:bj * 4 + 4, 8 * m + 4:8 * m + 8] = v[1]
    return occ


def kernel(_xyz, _scaling, _rotation, _opacity, resolution, num_blocks):
    assert int(resolution) == RES and int(num_blocks) == NB, \
        f"kernel hardcoded for resolution=64 num_blocks=16, got {resolution}/{num_blocks}"
    try:
        import concourse.bass_utils as bass_utils  # noqa: F401
    except ImportError:
        import sys
        sys.path.insert(0, "/opt/trn_rl_repo")
        import concourse.bass_utils as bass_utils

    _xyz = np.asarray(_xyz, np.float32)
    _scaling = np.asarray(_scaling, np.float32)
    _rotation = np.asarray(_rotation, np.float32)
    _opacity = np.asarray(_opacity, np.float32)

    xyzs, opa, logopa, inv = _host_prep(_xyz, _scaling, _rotation, _opacity)
    schedule, per_core = _build_workload(xyzs, opa, logopa, inv)
    in_maps, assembly = _build_inputs(schedule, per_core, xyzs, logopa, inv)

    key = (schedule["m_items"], tuple(schedule["l_sched"]))
    if key not in _CACHE:
        _CACHE.clear()
        _CACHE[key] = _build_program(schedule)
    nc = _CACHE[key]

    # the axon tunnel occasionally reports a transient
    # NRT_EXEC_UNIT_UNRECOVERABLE; it clears on retry
    import time
    last_err = None
    for attempt in range(4):
        try:
            res = bass_utils.run_bass_kernel_spmd(
                nc, in_maps, core_ids=list(range(N_CORES)))
            return _assemble(schedule, assembly, res.results)
        except Exception as e:  # noqa: BLE001
            last_err = e
            if "UNRECOVERABLE" not in str(e) and "UNAVAILABLE" not in str(e):
                raise
            time.sleep(10 * (attempt + 1))
    raise last_err



# revision 6
# speedup vs baseline: 1.2144x; 1.2144x over previous
"""Trainium2 Bass kernel for the GaussianModel occupancy-grid problem.

Strategy (v2: gaussian-major, matmul-reduce)
--------------------------------------------
occ[p] = sum_g w_g * exp(power(p, g)) with power a quadratic form in the
voxel coordinate p.  Per (gaussian, block) active pair ("slot"):

    power(p) = quad+lin(p_local) + const_g
    contribution = [w_g * e^{const_g}] * e^{quad+lin(p_local)}

The per-gaussian constant (and opacity weight) folds into a weight
w'_g = w_g e^{const_g}, so the device computes, per group of 128 slots:

    MM1 (PE):  power_ql[slot, vox] = coeff[18, 128slots]^T @ Phi[18, 64vox]
    ACT:       E = exp(power_ql)          (PSUM -> SBUF, fp32)
    MM2 (PE):  val[vox, piece] += E[slot, vox]^T @ w'[slot, piece]

Phi holds the 9 local-voxel features (x², y², z², xy, xz, yz, x, y, z)
scaled by 63 so every entry is a small odd-integer product — EXACT in
bf16.  Coefficients ship as bf16 hi + bf16 lo rows (rows 9..17 of Phi
duplicate rows 0..8), recovering fp32-level accuracy on the power while
keeping the 1-cycle/column bf16 matmul rate.  MM2 contracts over the
partition axis (slots), so the segmented gaussian reduction costs ~one
PE column per block piece and the vector engine is not on the critical
path at all.  Everything downstream of MM1 is fp32.

Blocks are dealt LPT-style across the 8 cores by active-gaussian count;
all cores run one SPMD program sized by the per-rank maxima.
"""

import numpy as np
import ml_dtypes

NB = 16          # num_blocks
RES = 64         # resolution
SPLIT = 4        # voxels per block side
N_CORES = 8
KF = 20          # 10 hi + 10 lo coefficient rows (incl. per-slot shift)
GRP = 128        # slots per group (MM2 contraction width)
VOX = 64         # voxels per block
TILE_G = 24      # groups per PSUM tile (24*64 = 1536 cols = 3 banks)

BF16 = ml_dtypes.bfloat16

_CACHE = {}


def _host_prep(_xyz, _scaling, _rotation, _opacity):
    """Mirror of the reference's per-gaussian preprocessing (numpy fp32)."""
    f32 = np.float32
    opac = (1.0 / (1.0 + np.exp(-_opacity[:, 0].astype(f32)))).astype(f32)
    keep = opac > 0.005
    opa = np.where(keep, opac, f32(0.0)).astype(f32)

    BIG = f32(1e10)
    mn = np.min(np.where(keep[:, None], _xyz, BIG), axis=0)
    mx = np.max(np.where(keep[:, None], _xyz, -BIG), axis=0)
    center = ((mn + mx) / 2).astype(f32)
    scale = (f32(1.8) / np.max(mx - mn)).astype(f32)
    xyzs = ((_xyz - center) * scale).astype(f32)
    stds = (np.exp(_scaling) * scale).astype(f32)

    q = (_rotation / np.linalg.norm(_rotation, axis=1, keepdims=True)).astype(f32)
    r, x, y, z = q[:, 0], q[:, 1], q[:, 2], q[:, 3]
    R = np.stack([
        np.stack([1 - 2 * (y * y + z * z), 2 * (x * y - r * z), 2 * (x * z + r * y)], -1),
        np.stack([2 * (x * y + r * z), 1 - 2 * (x * x + z * z), 2 * (y * z - r * x)], -1),
        np.stack([2 * (x * z - r * y), 2 * (y * z + r * x), 1 - 2 * (x * x + y * y)], -1),
    ], axis=1).astype(f32)
    L = R * stds[:, None, :]
    C = np.einsum('nij,nkj->nik', L, L).astype(f32)
    a, b, c = C[:, 0, 0], C[:, 0, 1], C[:, 0, 2]
    d, e, f = C[:, 1, 1], C[:, 1, 2], C[:, 2, 2]
    inv_det = (1.0 / (a * d * f + 2 * e * c * b - e * e * a - c * c * d
                      - b * b * f + 1e-24)).astype(f32)
    ia = ((d * f - e * e) * inv_det).astype(f32)
    ib = ((e * c - b * f) * inv_det).astype(f32)
    ic = ((e * b - c * d) * inv_det).astype(f32)
    id_ = ((a * f - c * c) * inv_det).astype(f32)
    ie = ((b * c - e * a) * inv_det).astype(f32)
    if_ = ((a * d - b * b) * inv_det).astype(f32)
    return xyzs, opa, (ia, ib, ic, id_, ie, if_)


def _build_workload(xyzs, opa):
    """Per-block active gaussian lists -> LPT core assignment -> group/piece
    packing and the shared SPMD schedule."""
    f32 = np.float32
    lin = np.linspace(-1.0, 1.0, RES).astype(f32)
    relax = f32((2.0 / NB) * 1.5)
    gx, gy, gz = xyzs[:, 0], xyzs[:, 1], xyzs[:, 2]
    act = opa > 0

    vmin = lin[np.arange(NB) * SPLIT] - relax
    vmax = lin[np.arange(NB) * SPLIT + SPLIT - 1] + relax
    Fx = (gx[None, :] > vmin[:, None]) & (gx[None, :] < vmax[:, None])
    Fy = (gy[None, :] > vmin[:, None]) & (gy[None, :] < vmax[:, None])
    Fz = ((gz[None, :] > vmin[:, None]) & (gz[None, :] < vmax[:, None])) & act

    blocks = []  # (n, bi, bj, bk, idx)
    for bi in range(NB):
        fx = Fx[bi]
        for bj in range(NB):
            fxy = fx & Fy[bj]
            if not fxy.any():
                continue
            for bk in range(NB):
                un = fxy & Fz[bk]
                idx = np.nonzero(un)[0]
                if idx.size:
                    blocks.append((idx.size, bi, bj, bk, idx))
    blocks.sort(key=lambda t: -t[0])

    # LPT deal by slot count
    loads = [0] * N_CORES
    core_blocks = [[] for _ in range(N_CORES)]
    for blk in blocks:
        c = min(range(N_CORES), key=lambda i: loads[i])
        core_blocks[c].append(blk)
        loads[c] += blk[0]

    # per-core group/piece packing
    per_core = []   # per core: list over groups of list of pieces
    g_counts = []
    for c in range(N_CORES):
        pieces_by_group = []
        cur = []       # pieces of current group
        fill = 0
        for (n, bi, bj, bk, idx) in core_blocks[c]:
            off = 0
            while off < n:
                take = min(n - off, GRP - fill)
                cur.append((bi, bj, bk, idx[off:off + take], fill))
                fill += take
                off += take
                if fill == GRP:
                    pieces_by_group.append(cur)
                    cur = []
                    fill = 0
        if cur:
            pieces_by_group.append(cur)
        per_core.append(pieces_by_group)
        g_counts.append(len(pieces_by_group))

    G = max(g_counts)
    nj = []
    for g in range(G):
        m = 0
        for c in range(N_CORES):
            if g < g_counts[c]:
                m = max(m, len(per_core[c][g]))
        nj.append(m)
    joff = np.concatenate([[0], np.cumsum(nj)]).astype(np.int64)
    J = int(joff[-1])
    assert J <= 1024, f"val columns {J} exceed 2 PSUM banks"

    schedule = {"G": G, "nj": tuple(nj), "joff": joff, "J": J,
                "T": (G + TILE_G - 1) // TILE_G}
    return schedule, per_core


def _build_inputs(schedule, per_core, xyzs, opa, inv):
    """Build phi/coeff/wind arrays per core + host assembly map."""
    f32 = np.float32
    ia, ib, ic, id_, ie, if_ = inv
    lin = np.linspace(-1.0, 1.0, RES).astype(f32)
    gx, gy, gz = xyzs[:, 0], xyzs[:, 1], xyzs[:, 2]
    G, joff, J = schedule["G"], schedule["joff"], schedule["J"]

    # Phi: 64 voxels of one block, p = ix*16 + iy*4 + iz; scaled coords
    # 63*(local offset) = odd ints {-3,-1,1,3} -> all features exact in bf16
    k = np.array([-3.0, -1.0, 1.0, 3.0], f32)
    X, Y, Z = np.meshgrid(k, k, k, indexing='ij')
    X, Y, Z = X.ravel(), Y.ravel(), Z.ravel()
    ones = np.ones(VOX, f32)
    feats = np.stack([X * X, Y * Y, Z * Z, X * Y, X * Z, Y * Z,
                      X, Y, Z, ones], 0)
    phi = np.zeros((KF, VOX), f32)
    phi[0:10] = feats
    phi[10:20] = feats
    phi_b = phi.astype(BF16)

    # local scale: voxel pitch is 2/63, offsets are k/2 * pitch = k/63
    s = f32(1.0) / f32(63.0)
    s2 = s * s

    logopa = np.where(opa > 0, np.log(np.maximum(opa, 1e-30)), f32(-1e10))

    in_maps = []
    assembly = []   # (core, col_j, bi, bj, bk)
    for c in range(N_CORES):
        coeff = np.zeros((10, G * GRP), f32)
        wind = np.zeros((GRP, J), f32)
        for g, pieces in enumerate(per_core[c]):
            for pi, (bi, bj, bk, idx, slot0) in enumerate(pieces):
                cx = f32((lin[bi * 4] + lin[bi * 4 + 3]) * 0.5)
                cy = f32((lin[bj * 4] + lin[bj * 4 + 3]) * 0.5)
                cz = f32((lin[bk * 4] + lin[bk * 4 + 3]) * 0.5)
                g0x = (gx[idx] - cx).astype(f32)
                g0y = (gy[idx] - cy).astype(f32)
                g0z = (gz[idx] - cz).astype(f32)
                A_ = ia[idx]; B_ = ib[idx]; Cc = ic[idx]
                D_ = id_[idx]; E_ = ie[idx]; F_ = if_[idx]
                Agx = A_ * g0x + B_ * g0y + Cc * g0z
                Agy = B_ * g0x + D_ * g0y + E_ * g0z
                Agz = Cc * g0x + E_ * g0y + F_ * g0z
                const = (-0.5 * (g0x * Agx + g0y * Agy + g0z * Agz)).astype(f32)
                # power_ql(p) <= -const at p=g; shift so the exp stays well
                # inside fp32 range and w' = opa*e^{const+shift} stays normal
                shift = np.maximum(0.0, -const - 60.0).astype(f32)

                o = g * GRP + slot0
                n = idx.size
                coeff[0, o:o + n] = -0.5 * A_ * s2
                coeff[1, o:o + n] = -0.5 * D_ * s2
                coeff[2, o:o + n] = -0.5 * F_ * s2
                coeff[3, o:o + n] = -B_ * s2
                coeff[4, o:o + n] = -Cc * s2
                coeff[5, o:o + n] = -E_ * s2
                coeff[6, o:o + n] = Agx * s
                coeff[7, o:o + n] = Agy * s
                coeff[8, o:o + n] = Agz * s
                coeff[9, o:o + n] = -shift
                wcol = (opa[idx] * np.exp(const + shift)).astype(f32)
                wind[slot0:slot0 + n, int(joff[g]) + pi] = wcol
                assembly.append((c, int(joff[g]) + pi, bi, bj, bk))
        hi = coeff.astype(BF16)
        lo = (coeff - hi.astype(f32)).astype(BF16)
        cfull = np.concatenate([hi, lo], axis=0)  # [20, G*128] bf16
        in_maps.append({"phi": phi_b, "coeff": cfull,
                        "wind": wind.astype(f32)})
    return in_maps, assembly


def _build_program(schedule):
    import concourse.bass as bass  # noqa: F401
    import concourse.bacc as bacc
    import concourse.tile as tile
    import concourse.mybir as mybir

    G = schedule["G"]
    nj = schedule["nj"]
    joff = schedule["joff"]
    J = schedule["J"]
    T = schedule["T"]
    f32 = mybir.dt.float32
    bf16 = mybir.dt.bfloat16

    nc = bacc.Bacc("TRN2", target_bir_lowering=False, debug=False,
                   num_devices=N_CORES)
    phi_d = nc.dram_tensor("phi", [KF, VOX], bf16, kind="ExternalInput")
    coeff_d = nc.dram_tensor("coeff", [KF, G * GRP], bf16, kind="ExternalInput")
    wind_d = nc.dram_tensor("wind", [GRP, J], f32, kind="ExternalInput")
    val_d = nc.dram_tensor("val", [VOX, J], f32, kind="ExternalOutput")

    tile_cols = TILE_G * VOX

    with tile.TileContext(nc) as tc:
        with tc.tile_pool(name="inp", bufs=1) as inp, \
             tc.tile_pool(name="es", bufs=3) as es, \
             tc.tile_pool(name="vs", bufs=1) as vs, \
             tc.tile_pool(name="ps", bufs=2, space="PSUM") as ps, \
             tc.tile_pool(name="vp", bufs=1, space="PSUM") as vp:
            phi_t = inp.tile([KF, VOX], bf16, name="phi_sb")
            coeff_t = inp.tile([KF, G * GRP], bf16, name="coeff_sb")
            wind_t = inp.tile([GRP, J], f32, name="wind_sb")
            val_sb = vs.tile([VOX, J], f32, name="val_sb")
            val_ps = vp.tile([VOX, J], f32, name="val_ps", tag="vp")

            nc.sync.dma_start(out=phi_t, in_=phi_d.ap())
            # coeff chunks sized to one PSUM tile's worth of groups so the
            # first matmuls aren't gated on the full load
            for t in range(T):
                c0 = t * TILE_G * GRP
                c1 = min(G * GRP, (t + 1) * TILE_G * GRP)
                nc.sync.dma_start(out=coeff_t[:, c0:c1],
                                  in_=coeff_d.ap()[:, c0:c1])
                if t == 0:
                    half = (J // 2 + 3) // 4 * 4
                    nc.sync.dma_start(out=wind_t[:, :half],
                                      in_=wind_d.ap()[:, :half])
                    nc.sync.dma_start(out=wind_t[:, half:],
                                      in_=wind_d.ap()[:, half:])

            ets = {}
            for t in range(T):
                g0, g1 = t * TILE_G, min(G, (t + 1) * TILE_G)
                used = (g1 - g0) * VOX
                pt = ps.tile([128, tile_cols], f32, name=f"pt{t}", tag="pt")
                for k, g in enumerate(range(g0, g1)):
                    nc.tensor.matmul(
                        pt[:, k * VOX:(k + 1) * VOX],
                        coeff_t[:, g * GRP:(g + 1) * GRP],
                        phi_t,
                        start=True, stop=True)
                et = es.tile([128, tile_cols], f32, name=f"et{t}", tag="et")
                ets[t] = et
                nc.scalar.activation(et[:, :used], pt[:, :used],
                                     mybir.ActivationFunctionType.Exp)
                # software pipelining: previous tile's MM2s go after this
                # tile's MM1s so the PE never blocks the next ACT
                if t > 0:
                    _emit_mm2s(nc, schedule, t - 1, ets[t - 1], wind_t, val_ps)
                    del ets[t - 1]
            _emit_mm2s(nc, schedule, T - 1, ets[T - 1], wind_t, val_ps)

            nc.vector.tensor_copy(out=val_sb, in_=val_ps)
            nc.sync.dma_start(out=val_d.ap(), in_=val_sb)

    nc.compile()
    return nc


def _emit_mm2s(nc, schedule, t, et, wind_t, val_ps):
    G = schedule["G"]
    nj = schedule["nj"]
    joff = schedule["joff"]
    g0, g1 = t * TILE_G, min(G, (t + 1) * TILE_G)
    for k, g in enumerate(range(g0, g1)):
        if nj[g] == 0:
            continue
        j0, j1 = int(joff[g]), int(joff[g]) + nj[g]
        nc.tensor.matmul(
            val_ps[:, j0:j1],
            et[:, k * VOX:(k + 1) * VOX],
            wind_t[:, j0:j1],
            start=True, stop=True)


def _assemble(schedule, assembly, results):
    occ = np.zeros((RES, RES, RES), np.float32)
    for (c, j, bi, bj, bk) in assembly:
        v = results[c]["val"][:, j].astype(np.float32).reshape(4, 4, 4)
        occ[bi * 4:bi * 4 + 4, bj * 4:bj * 4 + 4, bk * 4:bk * 4 + 4] += v
    return occ


def kernel(_xyz, _scaling, _rotation, _opacity, resolution, num_blocks):
    assert int(resolution) == RES and int(num_blocks) == NB, \
        f"kernel hardcoded for resolution=64 num_blocks=16, got {resolution}/{num_blocks}"
    try:
        import concourse.bass_utils as bass_utils  # noqa: F401
    except ImportError:
        import sys
        sys.path.insert(0, "/opt/trn_rl_repo")
        import concourse.bass_utils as bass_utils

    _xyz = np.asarray(_xyz, np.float32)
    _scaling = np.asarray(_scaling, np.float32)
    _rotation = np.asarray(_rotation, np.float32)
    _opacity = np.asarray(_opacity, np.float32)

    xyzs, opa, inv = _host_prep(_xyz, _scaling, _rotation, _opacity)
    schedule, per_core = _build_workload(xyzs, opa)
    in_maps, assembly = _build_inputs(schedule, per_core, xyzs, opa, inv)

    key = (schedule["G"], schedule["nj"])
    if key not in _CACHE:
        _CACHE.clear()
        _CACHE[key] = _build_program(schedule)
    nc = _CACHE[key]

    # the axon tunnel occasionally reports a transient
    # NRT_EXEC_UNIT_UNRECOVERABLE; it clears on retry
    import time
    last_err = None
    for attempt in range(4):
        try:
            res = bass_utils.run_bass_kernel_spmd(
                nc, in_maps, core_ids=list(range(N_CORES)))
            return _assemble(schedule, assembly, res.results)
        except Exception as e:  # noqa: BLE001
            last_err = e
            if "UNRECOVERABLE" not in str(e) and "UNAVAILABLE" not in str(e):
                raise
            time.sleep(10 * (attempt + 1))
    raise last_err
